# revision 12
# baseline (speedup 1.0000x reference)
"""Multi-head attention (B=2, S=2048, D=1024, H=16) on 8 Trainium2 NeuronCores.

Sharding (sequence-parallel; chosen over the hinted head-TP + all-reduce
because it needs only one small AllGather instead of a 16.8MB AllReduce):
  - B*S = 4096 token rows split 512/core; cores 0-3 own batch 0, 4-7 batch 1.
  - Each core: Q^T projection for its tokens; K^T projection for its tokens
    then AllGather within the 4-core batch group (2.1MB/core); V projection
    computed fully per-core (cheaper than a second AllGather and it fills the
    PE while the K AllGather is in flight), written straight into SBUF.
  - Full 16-head attention for the core's 512 query rows, then the row-slice
    of the output projection. Host output assembly is pure concatenation.

Layout: scores computed transposed (K @ Q^T per head) so exp-scores feed the
P*V matmul as the moving operand, the softmax denominator comes free via a
ones-column interleaved into V ([V_h | 1] stationary), and the P*V output
CT[d, q] is directly the stationary operand of the output projection. The
per-q normalization is broadcast across partitions with a K=1 ones matmul.

PE scheduling: QK matmuls for the two heads of a pair alternate between
array row-groups 0-63 / 64-127 (KT tiles hold head pairs on the partition
axis), which lets consecutive dk=64 matmuls overlap in disjoint array halves
(~139ns vs 427ns measured). P*V (K=128) runs at full rate.

dtype: float32r matmuls (~1.6e-4/matmul), fp32 psum + softmax. Reference's
max-subtraction is an exact no-op (scores ~N(0,1)) and is skipped; its +1e-9
on the denominator is below f32r resolution (denom >= 1).
"""

import sys

if "/opt/trn_rl_repo" not in sys.path:
    sys.path.insert(0, "/opt/trn_rl_repo")

import numpy as np

B, S, D = 2, 2048, 1024
H, DK = 16, 64
N_CORES = 8
DT = B * S // N_CORES          # 512 tokens per core
NB = 4                         # cores per batch group
KB = NB * DT                   # 2048 keys per batch group
NJ = KB // 128                 # 16 key tiles
GROUPS = [[0, 1, 2, 3], [4, 5, 6, 7]]

_CACHE = {}


def _build():
    import concourse.bass as bass
    import concourse.bacc as bacc
    import concourse.mybir as mybir
    import concourse.tile as tile
    from contextlib import ExitStack

    f32 = mybir.dt.float32
    f32r = mybir.dt.float32r
    EXP = mybir.ActivationFunctionType.Exp

    nc = bacc.Bacc("TRN2", target_bir_lowering=False, debug=False,
                   num_devices=N_CORES)

    # ---- I/O ----
    qT = nc.dram_tensor("qT", [D, DT], f32r, kind="ExternalInput")
    kT = nc.dram_tensor("kT", [D, DT], f32r, kind="ExternalInput")
    vT = nc.dram_tensor("vT", [D, KB], f32r, kind="ExternalInput")  # full batch
    wqT = nc.dram_tensor("wqT", [D, D], f32r, kind="ExternalInput")
    wkT = nc.dram_tensor("wkT", [D, D], f32r, kind="ExternalInput")
    wvT = nc.dram_tensor("wvT", [D, D], f32r, kind="ExternalInput")
    woT = nc.dram_tensor("woT", [D, D], f32r, kind="ExternalInput")
    bq = nc.dram_tensor("bq", [D], f32, kind="ExternalInput")
    bk = nc.dram_tensor("bk", [D], f32, kind="ExternalInput")
    bv = nc.dram_tensor("bv", [D], f32r, kind="ExternalInput")
    bo = nc.dram_tensor("bo", [D], f32r, kind="ExternalInput")
    onesin = nc.dram_tensor("onesin", [128, 128], f32r, kind="ExternalInput")
    out = nc.dram_tensor("out", [DT, D], f32, kind="ExternalOutput")

    with tile.TileContext(nc) as tc, ExitStack() as top:
        # ---- long-lived tiles ----
        const = top.enter_context(tc.tile_pool(name="const", bufs=1))
        ones_sb = const.tile([128, 128], f32r, tag="ones")
        nc.sync.dma_start(ones_sb[:], onesin.ap())
        bq_sb = const.tile([128, 8], f32, tag="bq")
        nc.sync.dma_start(bq_sb[:], bq.ap().rearrange("(a p) -> p a", p=128))
        bk_sb = const.tile([128, 8], f32, tag="bk")
        nc.sync.dma_start(bk_sb[:], bk.ap().rearrange("(a p) -> p a", p=128))
        bv_sb = const.tile([1, D], f32r, tag="bv")
        nc.sync.dma_start(bv_sb[:], bv.ap().rearrange("(a d) -> a d", a=1))
        bo_sb = const.tile([1, D], f32r, tag="bo")
        nc.sync.dma_start(bo_sb[:], bo.ap().rearrange("(a d) -> a d", a=1))

        qt_pool = top.enter_context(tc.tile_pool(name="qt", bufs=1))
        qt_sb = [qt_pool.tile([128, DT], f32r, tag=f"qt{s}", name=f"qt{s}")
                 for s in range(8)]
        ct_pool = top.enter_context(tc.tile_pool(name="ct", bufs=1))
        ct_sb = [ct_pool.tile([128, DT], f32r, tag=f"ct{t}", name=f"ct{t}")
                 for t in range(8)]
        # V with interleaved ones columns: [128, 16*(64+1)] per key tile
        vpl_pool = top.enter_context(tc.tile_pool(name="vpl", bufs=1))
        vp = [vpl_pool.tile([128, H * (DK + 1)], f32r, tag=f"vp{j}",
                            name=f"vp{j}") for j in range(NJ)]

        # PSUM budget (8 banks): pss 2x[128,1024]=4, pct 2x[65,512]=2,
        # pb 1x[128,512]=1, pp 1x[128,512]=1 (projections + O-proj).
        ps_s = top.enter_context(tc.tile_pool(name="ps_s", bufs=2, space="PSUM"))
        ps_cb = top.enter_context(tc.tile_pool(name="ps_cb", bufs=1, space="PSUM"))
        psp = top.enter_context(tc.tile_pool(name="psp", bufs=1, space="PSUM"))

        # HAM warm-up: a dense ~5us burst of dummy matmuls so the PE clock
        # reaches 2.4GHz before the real stream starts.
        wu = ps_cb.tile([128, 128], f32, tag="pb", name="wu", bufs=1)
        for _ in range(48):
            nc.tensor.matmul(wu[:], ones_sb[:], ones_sb[:],
                             start=True, stop=True)

        dram = top.enter_context(tc.tile_pool(name="dram", bufs=1, space="DRAM"))
        ktp_in = dram.tile([D, DT], f32r, tag="ktp_in", name="ktp_in")
        ktp_out = dram.tile([NB * D, DT], f32r, tag="ktp_out", name="ktp_out")

        # ---- phase B: K^T projection (sharded) + AllGather ----
        # DMA issue is spread across engine queues: sync owns the K/Q path,
        # gpsimd (SWDGE) owns the V path, vector prefetches attention tiles.
        with ExitStack() as ph:
            inp = ph.enter_context(tc.tile_pool(name="inp", bufs=1))
            wpool = ph.enter_context(tc.tile_pool(name="wpool", bufs=8))
            stg = ph.enter_context(tc.tile_pool(name="stg", bufs=3))

            kin = [inp.tile([128, DT], f32r, tag=f"kin{i}", name=f"kin{i}")
                   for i in range(8)]
            for i in range(8):
                nc.sync.dma_start(kin[i][:], kT.ap()[i * 128:(i + 1) * 128, :])

            def proj_qk(win, xin, bias_sb, evict):
                # weight chunks [128, 512] each cover 4 dout slices
                for sg in range(2):
                    wch = []
                    for i in range(8):
                        w = wpool.tile([128, 512], f32r, tag="w", name="w")
                        nc.sync.dma_start(
                            w[:], win.ap()[i * 128:(i + 1) * 128,
                                           sg * 512:(sg + 1) * 512])
                        wch.append(w)
                    for s in range(sg * 4, sg * 4 + 4):
                        pp = psp.tile([128, DT], f32, tag="pp", name="pp")
                        for i in range(8):
                            nc.tensor.matmul(
                                pp[:], wch[i][:, (s % 4) * 128:(s % 4 + 1) * 128],
                                xin[i][:], start=(i == 0), stop=(i == 7))
                        evict(s, pp, bias_sb)

            def k_evict(s, pp, bias_sb):
                st = stg.tile([128, DT], f32r, tag="st", name="st")
                nc.scalar.add(st[:], pp[:], bias_sb[:, s:s + 1])
                nc.sync.dma_start(ktp_in[s * 128:(s + 1) * 128, :], st[:])

            proj_qk(wkT, kin, bk_sb, k_evict)

            nc.gpsimd.collective_compute(
                "AllGather", mybir.AluOpType.bypass, replica_groups=GROUPS,
                ins=[ktp_in.opt()], outs=[ktp_out.opt()])

            wvpool = ph.enter_context(tc.tile_pool(name="wvpool", bufs=1))
            vinp = ph.enter_context(tc.tile_pool(name="vinp", bufs=9))

            # ones columns of vp, on vector
            for j in range(NJ):
                oc = vp[j][:].rearrange("p (h d) -> p h d", h=H)[:, :, DK:DK + 1]
                nc.gpsimd.dma_start(
                    oc, onesin.ap().rearrange("p (h d) -> p h d", h=H)[:, :, 0:1])

            # Q^T projection (overlaps the collective)
            # qin reuses kin's pool slots (kin is dead after the K projection)
            qin = [inp.tile([128, DT], f32r, tag=f"kin{i}", name=f"qin{i}")
                   for i in range(8)]
            for i in range(8):
                nc.sync.dma_start(qin[i][:], qT.ap()[i * 128:(i + 1) * 128, :])

            def q_evict(s, pp, bias_sb):
                nc.scalar.add(qt_sb[s][:], pp[:], bias_sb[:, s:s + 1])

            proj_qk(wqT, qin, bq_sb, q_evict)

            # V projection, full batch (2048 keys), written straight into vp.
            # Per dout-half: 8 weight tiles live; vin chunks re-streamed.
            for dh in range(2):
                wvd = []
                for i in range(8):
                    wt = wvpool.tile([128, 512], f32r, tag=f"wv{i}",
                                     name=f"wv{i}")
                    nc.gpsimd.dma_start(
                        wt[:], wvT.ap()[i * 128:(i + 1) * 128,
                                        dh * 512:(dh + 1) * 512])
                    wvd.append(wt)
                for jh in range(2):
                    vin = []
                    for i in range(8):
                        vt = vinp.tile([128, 8 * 128], f32r, tag="vin",
                                       name="vin")
                        nc.gpsimd.dma_start(
                            vt[:], vT.ap()[i * 128:(i + 1) * 128,
                                           jh * 1024:(jh + 1) * 1024])
                        vin.append(vt)
                    for jl in range(8):
                        j = jh * 8 + jl
                        pp = psp.tile([128, 512], f32, tag="pp", name="ppv")
                        for i in range(8):
                            nc.tensor.matmul(
                                pp[:], vin[i][:, jl * 128:(jl + 1) * 128],
                                wvd[i][:], start=(i == 0), stop=False)
                        nc.tensor.matmul(pp[:], ones_sb[0:1, :],
                                         bv_sb[0:1, dh * 512:(dh + 1) * 512],
                                         start=False, stop=True)
                        # evict into interleaved [V_h | 1] layout
                        dst = vp[j][:, 8 * dh * 65:(8 * dh + 8) * 65].rearrange(
                            "p (h d) -> p h d", h=8)[:, :, 0:DK]
                        with nc.allow_low_precision(reason="f32r V evict"):
                            nc.vector.tensor_copy(dst, pp[:].rearrange(
                                "p (h d) -> p h d", h=8))

        # ---- phase D: attention ----
        with ExitStack() as ph:
            ktf_pool = ph.enter_context(tc.tile_pool(name="ktf", bufs=8))
            ex_pool = ph.enter_context(tc.tile_pool(name="ex", bufs=3))
            rc_pool = ph.enter_context(tc.tile_pool(name="rc", bufs=2))
            bc_pool = ph.enter_context(tc.tile_pool(name="bc", bufs=2))

            for t in range(8):                      # head pairs
                ktf = []
                for r in range(NB):
                    kt_t = ktf_pool.tile([128, DT], f32r, tag="ktf", name="ktf")
                    nc.sync.dma_start(
                        kt_t[:], ktp_out[r * D + t * 128: r * D + (t + 1) * 128, :])
                    ktf.append(kt_t)
                pct = [ps_cb.tile([65, DT], f32, tag="pct", name="pct",
                                  bufs=2) for _ in range(2)]
                for j in range(NJ):
                    r, jl = j // NB, j % NB
                    pss = ps_s.tile([128, 2 * DT], f32, tag="pss", name="pss")
                    ex = ex_pool.tile([128, 2 * DT], f32r, tag="ex", name="ex")
                    # the two heads alternate array row-groups 0-63 / 64-127
                    for hh in range(2):
                        nc.tensor.matmul(
                            pss[:, hh * DT:(hh + 1) * DT],
                            ktf[r][hh * 64:hh * 64 + 64, jl * 128:(jl + 1) * 128],
                            qt_sb[t][hh * 64:hh * 64 + 64, :],
                            start=True, stop=True)
                    nc.scalar.activation(ex[:], pss[:], EXP, scale=0.125)
                    for hh in range(2):
                        h = 2 * t + hh
                        nc.tensor.matmul(
                            pct[hh][:], vp[j][:, h * 65:(h + 1) * 65],
                            ex[:, hh * DT:(hh + 1) * DT],
                            start=(j == 0), stop=(j == NJ - 1))
                for hh in range(2):
                    # NOTE: custom-DVE ops (reciprocal_approx_*) return garbage
                    # for partition-offset PSUM inputs; plain reciprocal works.
                    rc = rc_pool.tile([1, DT], f32r, tag="rc", name="rc")
                    with nc.allow_low_precision(reason="f32r softmax recip"):
                        nc.vector.reciprocal(rc[:], pct[hh][64:65, :])
                    rcr = rc[:]
                    pb = ps_cb.tile([128, DT], f32, tag="pb", name="pb",
                                    bufs=1)
                    nc.tensor.matmul(pb[:], ones_sb[0:1, :], rcr,
                                     start=True, stop=True)
                    bcst = bc_pool.tile([128, DT], f32, tag="bc", name="bc")
                    nc.vector.tensor_copy(bcst[:], pb[:])
                    with nc.allow_low_precision(reason="f32r ctx normalize"):
                        nc.vector.tensor_mul(
                            ct_sb[t][hh * 64:hh * 64 + 64, :],
                            pct[hh][0:64, :], bcst[0:64, :])

        # ---- phase E: output projection ----
        with ExitStack() as ph:
            wo_pool = ph.enter_context(tc.tile_pool(name="wo", bufs=16))
            ob_pool = ph.enter_context(tc.tile_pool(name="ob", bufs=3))
            for dh in range(2):
                wo = []
                for tt in range(8):
                    wt = wo_pool.tile([128, 512], f32r, tag="wo", name="wo")
                    nc.sync.dma_start(
                        wt[:], woT.ap()[tt * 128:(tt + 1) * 128,
                                        dh * 512:(dh + 1) * 512])
                    wo.append(wt)
                for qs in range(4):
                    po = psp.tile([128, 512], f32, tag="pp", name="po")
                    for tt in range(8):
                        nc.tensor.matmul(po[:],
                                         ct_sb[tt][:, qs * 128:(qs + 1) * 128],
                                         wo[tt][:], start=(tt == 0), stop=False)
                    nc.tensor.matmul(po[:], ones_sb[0:1, :],
                                     bo_sb[0:1, dh * 512:(dh + 1) * 512],
                                     start=False, stop=True)
                    ob = ob_pool.tile([128, 512], f32, tag="ob", name="ob")
                    nc.scalar.copy(ob[:], po[:])
                    nc.sync.dma_start(
                        out.ap()[qs * 128:(qs + 1) * 128,
                                 dh * 512:(dh + 1) * 512], ob[:])

    nc.compile()
    return nc


def _get_nc():
    if "nc" not in _CACHE:
        _CACHE["nc"] = _build()
    return _CACHE["nc"]


def _prep_in_maps(q, k, v, Wq, bq, Wk, bk, Wv, bv, Wo, bo):
    f = lambda a: np.ascontiguousarray(np.asarray(a, dtype=np.float32))
    qT = f(np.asarray(q, dtype=np.float32).reshape(B * S, D).T)
    kT = f(np.asarray(k, dtype=np.float32).reshape(B * S, D).T)
    vT = f(np.asarray(v, dtype=np.float32).reshape(B * S, D).T)
    shared = {
        "wqT": f(np.asarray(Wq).T), "wkT": f(np.asarray(Wk).T),
        "wvT": f(np.asarray(Wv).T), "woT": f(np.asarray(Wo).T),
        "bq": f(bq), "bk": f(bk), "bv": f(bv), "bo": f(bo),
        "onesin": np.ones((128, 128), np.float32),
    }
    in_maps = []
    for c in range(N_CORES):
        g = c // NB
        in_maps.append({
            "qT": np.ascontiguousarray(qT[:, c * DT:(c + 1) * DT]),
            "kT": np.ascontiguousarray(kT[:, c * DT:(c + 1) * DT]),
            "vT": np.ascontiguousarray(vT[:, g * KB:(g + 1) * KB]),
            **shared,
        })
    return in_maps


def _run(in_maps, trace=False, **kw):
    from concourse.bass_utils import run_bass_kernel_spmd
    nc = _get_nc()
    res = run_bass_kernel_spmd(nc, in_maps, core_ids=list(range(N_CORES)),
                               trace=trace, **kw)
    full = np.concatenate([res.results[c]["out"] for c in range(N_CORES)],
                          axis=0).reshape(B, S, D)
    return full, res


def kernel(q, k, v, Wq, bq, Wk, bk, Wv, bv, Wo, bo):
    in_maps = _prep_in_maps(q, k, v, Wq, bq, Wk, bk, Wv, bv, Wo, bo)
    full, _ = _run(in_maps, trace=False)
    return full
